# revision 1
# baseline (speedup 1.0000x reference)
"""HGRNet (hypergraph conv net) kernel for 8 trn2 NeuronCores.

Structure: the two HyConv layers are h = lrelu(D^-1 (I+S)(I+S^T)/K (x @ theta) + b).
The dense matmuls (x @ theta0: 10.5 GFLOP, h1 @ theta1: 2.6 GFLOP) run on the
8 NeuronCores, data-parallel per the sharding hint: 4 slides x 2 token-halves,
each core computing a (5056, Cout) strip of its slide. The sparse segment-sum
message passing runs host-side with sorted-CSR reduceat (this axon terminal's
runtime rejects indirect/gather DMA: InstDMAGatherAnt and
InstDMACopy+dynamic_ap_info both abort at execute with INTERNAL — verified).

Device kernel: strip-batched loads (512KB DMAs), 8 PSUM banks per group,
batched strided output writes. Cost-model (TimelineSim): 85us (L1) + 36us
(L2) per launch, PE-bound (fp32 matmul); the naive per-tile version was
280+168us (HWDGE per-DMA fixed cost dominated). Rejected variants: a
weight-stationary transposed layout (92+43us, slower in the cost model);
float32r matmul inputs (57+31us modeled = 27% faster, compiles and runs
when tensors are declared f32r natively rather than bitcast, but HW-measured
matmul error is ~1.5e-4 rel — f32r is a reduced-precision streaming mode on
silicon, so it was dropped to preserve the 1.5e-6 fp32 accuracy); bf16
(2x PE, same accuracy objection); buffer/group-size sweeps (+-1%, plateau).

Self-contained: hardcodes B=4, N=10000, Cin=512, H1=256, H2=128, T=64, K=10.
"""
import numpy as np

K = 10
NEG_SLOPE = 0.01
B, N, CIN, H1, H2, T = 4, 10000, 512, 256, 128, 64
NCORES = 8
NHALF = 5056          # tokens per core (2 halves x 4 slides)
NHPAD = 5120          # 40 * 128

_cache = {}


def _build_matmul_nc(cin, cout):
    """SPMD program: out[tok, :] = xT[:, tok].T @ w  for NHPAD tokens.

    xT: (cin, NHPAD) f32, w: (cin, cout) f32, out: (NHPAD, cout) f32.
    cin % 128 == 0, cout <= 512.
    """
    from concourse import bacc, tile, mybir

    nc = bacc.Bacc("TRN2", target_bir_lowering=False, debug=False,
                   enable_asserts=True, num_devices=NCORES)
    xT = nc.dram_tensor("xT", [cin, NHPAD], mybir.dt.float32, kind="ExternalInput")
    w = nc.dram_tensor("w", [cin, cout], mybir.dt.float32, kind="ExternalInput")
    out = nc.dram_tensor("out", [NHPAD, cout], mybir.dt.float32,
                         kind="ExternalOutput")

    kc = cin // 128
    ntile = NHPAD // 128
    GRP = 8  # token tiles per group = 8 PSUM banks

    with tile.TileContext(nc) as tc:
        with (
            tc.tile_pool(name="wp", bufs=1) as wp,
            tc.tile_pool(name="xp", bufs=2 * kc) as xp,
            tc.tile_pool(name="op", bufs=2) as op,
            tc.tile_pool(name="ps", bufs=GRP, space="PSUM") as ps,
        ):
            w_tiles = []
            for k in range(kc):
                wt = wp.tile([128, cout], mybir.dt.float32, tag=f"w{k}")
                nc.sync.dma_start(wt[:], w[128 * k:128 * (k + 1), :])
                w_tiles.append(wt)
            t0 = 0
            while t0 < ntile:
                g = min(GRP, ntile - t0)
                strips = []
                for k in range(kc):
                    st = xp.tile([128, GRP * 128], mybir.dt.float32, tag="strip")
                    nc.sync.dma_start(
                        st[:, :g * 128],
                        xT[128 * k:128 * (k + 1), 128 * t0:128 * (t0 + g)])
                    strips.append(st)
                ot = op.tile([128, GRP, cout], mybir.dt.float32, tag="o")
                for j in range(g):
                    pt = ps.tile([128, cout], mybir.dt.float32)
                    for k in range(kc):
                        nc.tensor.matmul(
                            pt[:], strips[k][:, 128 * j:128 * (j + 1)],
                            w_tiles[k][:], start=(k == 0), stop=(k == kc - 1))
                    nc.vector.tensor_copy(ot[:, j, :], pt[:])
                nc.sync.dma_start(
                    out.ap().rearrange("(t p) c -> p t c", p=128)[:, t0:t0 + g, :],
                    ot[:, :g, :])
                t0 += g
    nc.compile()
    return nc


def _get_nc(cin, cout):
    if (cin, cout) not in _cache:
        _cache[(cin, cout)] = _build_matmul_nc(cin, cout)
    return _cache[(cin, cout)]


def _device_linear(xf, w):
    """xf: (B, N, cin) -> (B, N, cout) via 8-core SPMD (4 slides x 2 halves)."""
    from concourse.bass_utils import run_bass_kernel_spmd

    cin, cout = w.shape
    nc = _get_nc(cin, cout)
    w = np.ascontiguousarray(w, np.float32)
    in_maps = []
    for c in range(NCORES):
        b, h = c // 2, c % 2
        t0 = h * NHALF
        ln = min(NHALF, N - t0)
        xTh = np.zeros((cin, NHPAD), np.float32)
        xTh[:, :ln] = xf[b, t0:t0 + ln].T
        in_maps.append({"xT": xTh, "w": w})
    res = run_bass_kernel_spmd(nc, in_maps, core_ids=list(range(NCORES)))
    out = np.empty((B, N, cout), np.float32)
    for c in range(NCORES):
        b, h = c // 2, c % 2
        t0 = h * NHALF
        ln = min(NHALF, N - t0)
        out[b, t0:t0 + ln] = res.results[c]["out"][:ln]
    return out


def _segment_csr(samp):
    """Sorted-CSR for the scatter direction of one slide. samp: (N, K-1)."""
    flat = samp.ravel()
    order = np.argsort(flat, kind="stable")
    src_sorted = (order // (K - 1)).astype(np.int64)
    counts = np.bincount(flat, minlength=N)
    starts = np.zeros(N, np.int64)
    np.cumsum(counts[:-1], out=starts[1:])
    return src_sorted, starts, counts


def _hyconv_host(xt, samp, csr, rdv, bias):
    """lrelu(rdv * (ef + scatter(ef)) + bias), ef = mean over hyperedge."""
    src_sorted, starts, counts = csr
    ef = (xt + xt[samp].sum(axis=1)) * (1.0 / K)       # (N, C)
    contrib = np.add.reduceat(ef[src_sorted], starts, axis=0)
    contrib[counts == 0] = 0.0
    pre = (ef + contrib) * rdv[:, None] + bias
    return np.where(pre > 0, pre, NEG_SLOPE * pre).astype(np.float32)


def kernel(x, nn_idx, theta0, b0, theta1, b1, fc_w, fc_b):
    x = np.asarray(x, np.float32)
    nn_idx = np.asarray(nn_idx).astype(np.int64)
    theta0 = np.asarray(theta0, np.float32)
    b0 = np.asarray(b0, np.float32)
    theta1 = np.asarray(theta1, np.float32)
    b1 = np.asarray(b1, np.float32)
    fc_w = np.asarray(fc_w, np.float32)
    fc_b = np.asarray(fc_b, np.float32)

    perm = np.random.RandomState(0).permutation(2 * K - 1)[:K - 1]
    samps = [nn_idx[b][:, perm] for b in range(B)]
    csrs = [_segment_csr(s) for s in samps]
    rdvs = [1.0 / np.maximum(csrs[b][2] + 1.0, 1.0).astype(np.float32)
            for b in range(B)]

    xt0 = _device_linear(x, theta0)
    h1 = np.stack([
        _hyconv_host(xt0[b], samps[b], csrs[b], rdvs[b], b0) for b in range(B)])
    xt1 = _device_linear(h1, theta1)
    h2 = np.stack([
        _hyconv_host(xt1[b], samps[b], csrs[b], rdvs[b], b1) for b in range(B)])
    pooled = h2.mean(axis=1)
    return (pooled @ fc_w + fc_b).astype(np.float32)

